# revision 1
# baseline (speedup 1.0000x reference)
"""Trainium2 Bass kernel for nn_ContrastiveLoss (B=4096, D=512, 8 cores).

Strategy (data-parallel over the 2B=8192 rows of reps = [emb_i; emb_j]):
  - Host passes X.T (D-major) to every core plus a per-core 1024-column row
    block (qt) and its positive-partner block (pt).
  - On device, column norms of X.T are computed with ones-vector matmuls
    (partition-dim reduction on the PE), columns are normalized in place,
    and each core computes its (1024 x 8192) block of the similarity matrix
    as qt.T @ zt with float32r (full-rate) matmuls, fusing exp(sim/t) and
    the row-sum into ScalarE activations reading PSUM directly.
  - The fu term (rowwise dot(z_k, z_i)) is computed redundantly on every
    core; the final per-row -log(nom/denom) reduces to a [128,1] partial
    per core which the host sums.
"""

import numpy as np

import concourse.bass as bass
import concourse.mybir as mybir
import concourse.tile as tile
from concourse import bacc

f32 = mybir.dt.float32
f32r = mybir.dt.float32r
AF = mybir.ActivationFunctionType
OP = mybir.AluOpType
AX = mybir.AxisListType

P = 128
TEMP = 0.2
INV_T = 1.0 / TEMP  # 5.0


def build_nc(two_n=8192, d=512, q=1024, b_fu=4096, dbg=False):
    """Build the SPMD Bass program (identical on all cores; data differs)."""
    assert two_n % 512 == 0 and d % P == 0 and q % P == 0 and b_fu % 512 == 0
    DT = d // P          # d-tiles (contraction)
    NT = two_n // 512    # column groups of 512
    MT = q // P          # m-tiles (output rows per core / 128)
    FT = b_fu // 512     # fu column slices
    QC = [(i * 512, min(512, q - i * 512)) for i in range((q + 511) // 512)]

    nc = bacc.Bacc("TRN2", target_bir_lowering=False, debug=False)

    xt_d = nc.dram_tensor("xt", [d, two_n], f32r, kind="ExternalInput")
    qt_d = nc.dram_tensor("qt", [d, q], f32r, kind="ExternalInput")
    pt_d = nc.dram_tensor("pt", [d, q], f32r, kind="ExternalInput")
    kt_d = nc.dram_tensor("kt", [d, b_fu], f32r, kind="ExternalInput")
    ones_d = nc.dram_tensor("ones", [P, P], f32r, kind="ExternalInput")
    out_d = nc.dram_tensor("partial", [P, 1], f32, kind="ExternalOutput")
    if dbg:
        dbg_d = {name: nc.dram_tensor(f"dbg_{name}", shape, f32, kind="ExternalOutput")
                 for name, shape in [
                     ("n2q", [P, q // P]), ("invq", [P, q // P]),
                     ("pos_t", [P, q // P]), ("selfexp", [P, q // P]),
                     ("slots", [P, (q // P) * (two_n // 512)]),
                     ("rs_all", [P, q // P]), ("denom", [P, q // P]),
                     ("fu_parts", [1, max(b_fu // 512, 2)]),
                     ("bc0", [P, 512]), ("g00", [P, 512]),
                 ]}
    fu_scr = nc.dram_tensor("fu_scr", [1, 1], f32)  # bounce for fu broadcast
    row_scr = nc.dram_tensor("row_scr", [3, q], f32)  # bounce for row reshapes

    with tile.TileContext(nc) as tc:
        with (
            tc.tile_pool(name="xp", bufs=1) as xp,
            tc.tile_pool(name="qp", bufs=1) as qp,
            tc.tile_pool(name="stream", bufs=2) as stream,   # pt/kt chunks
            tc.tile_pool(name="sqp", bufs=3) as sqp,         # squares/products
            tc.tile_pool(name="bcp", bufs=2) as bcp,         # bcast inv tiles
            tc.tile_pool(name="scrp", bufs=3) as scrp,       # exp main outputs
            tc.tile_pool(name="rowp", bufs=4) as rowp,       # [1,512] pieces
            tc.tile_pool(name="sm", bufs=1) as sm,           # persistent smalls
            tc.tile_pool(name="psg", bufs=4, space="PSUM") as psg,
            tc.tile_pool(name="psb", bufs=1, space="PSUM") as psb,
            tc.tile_pool(name="pss", bufs=3, space="PSUM") as pss,
        ):
            ones_col = sm.tile([P, 1], f32r, tag="ones_col")
            nc.gpsimd.dma_start(ones_col[:], ones_d[:, 0:1])
            ones_row = sm.tile([1, P], f32r, tag="ones_row")
            nc.gpsimd.dma_start(ones_row[:], ones_d[0:1, :])

            # ---- load qt (lhsT blocks, raw) ----
            qt_sb = []
            for dt in range(DT):
                t = qp.tile([P, q], f32r, tag=f"qt{dt}")
                nc.gpsimd.dma_start(t[:], qt_d[dt * P:(dt + 1) * P, :])
                qt_sb.append(t)

            # ---- qt column norms^2 -> n2q_row [1, q] ----
            n2q_row = sm.tile([1, q], f32, tag="n2q_row")
            for (c0, cw) in QC:
                ps = pss.tile([1, 512], f32, tag="small")
                for dt in range(DT):
                    sq = sqp.tile([P, 512], f32r, tag="sq")
                    nc.vector.tensor_mul(
                        sq[:, :cw], qt_sb[dt][:, c0:c0 + cw], qt_sb[dt][:, c0:c0 + cw])
                    nc.tensor.matmul(ps[:, :cw], ones_col[:], sq[:, :cw],
                                     start=(dt == 0), stop=(dt == DT - 1))
                nc.scalar.activation(n2q_row[0:1, c0:c0 + cw], ps[:, :cw], AF.Copy)

            # ---- pt stats: n2p_row and posr_row (rowwise dot q.p) ----
            n2p_row = sm.tile([1, q], f32, tag="n2p_row")
            posr_row = sm.tile([1, q], f32, tag="posr_row")
            for (c0, cw) in QC:
                ps_p2 = pss.tile([1, 512], f32, tag="small")
                ps_pr = pss.tile([1, 512], f32, tag="small")
                for dt in range(DT):
                    ptc = stream.tile([P, 512], f32r, tag="pt")
                    nc.gpsimd.dma_start(ptc[:, :cw], pt_d[dt * P:(dt + 1) * P, c0:c0 + cw])
                    sq = sqp.tile([P, 512], f32r, tag="sq")
                    nc.vector.tensor_mul(sq[:, :cw], ptc[:, :cw], ptc[:, :cw])
                    nc.tensor.matmul(ps_p2[:, :cw], ones_col[:], sq[:, :cw],
                                     start=(dt == 0), stop=(dt == DT - 1))
                    qp_ = sqp.tile([P, 512], f32r, tag="sq")
                    nc.vector.tensor_mul(
                        qp_[:, :cw], qt_sb[dt][:, c0:c0 + cw], ptc[:, :cw])
                    nc.tensor.matmul(ps_pr[:, :cw], ones_col[:], qp_[:, :cw],
                                     start=(dt == 0), stop=(dt == DT - 1))
                nc.scalar.activation(n2p_row[0:1, c0:c0 + cw], ps_p2[:, :cw], AF.Copy)
                nc.scalar.activation(posr_row[0:1, c0:c0 + cw], ps_pr[:, :cw], AF.Copy)

            # ---- reshape rows -> [P, MT] tiles; [p, m] = row[m*128 + p] ----
            # SBUF->SBUF partition-scatter DMAs corrupt on HW; bounce via DRAM
            # (DRAM->SBUF strided loads are the standard safe pattern).
            nc.gpsimd.dma_start(row_scr[0:1, :], n2q_row[:])
            nc.gpsimd.dma_start(row_scr[1:2, :], n2p_row[:])
            nc.gpsimd.dma_start(row_scr[2:3, :], posr_row[:])
            n2q = sm.tile([P, MT], f32, tag="n2q")
            nc.gpsimd.dma_start(
                n2q[:], row_scr[0:1, :].rearrange("a (m p) -> (a p) m", p=P))
            n2p = sm.tile([P, MT], f32, tag="n2p")
            nc.gpsimd.dma_start(
                n2p[:], row_scr[1:2, :].rearrange("a (m p) -> (a p) m", p=P))
            posr = sm.tile([P, MT], f32, tag="posr")
            nc.gpsimd.dma_start(
                posr[:], row_scr[2:3, :].rearrange("a (m p) -> (a p) m", p=P))

            tmp = sm.tile([P, MT], f32, tag="tmp")
            invq = sm.tile([P, MT], f32, tag="invq")
            nc.scalar.activation(tmp[:], n2q[:], AF.Sqrt)
            nc.vector.reciprocal(invq[:], tmp[:])
            invp = sm.tile([P, MT], f32, tag="invp")
            nc.scalar.activation(tmp[:], n2p[:], AF.Sqrt)
            nc.vector.reciprocal(invp[:], tmp[:])
            invq_t = sm.tile([P, MT], f32, tag="invq_t")
            nc.vector.tensor_scalar_mul(invq_t[:], invq[:], INV_T)

            # selfexp = exp(n2q * invq^2 / t)
            self_t = sm.tile([P, MT], f32, tag="self_t")
            nc.vector.tensor_mul(self_t[:], n2q[:], invq[:])
            nc.vector.tensor_mul(self_t[:], self_t[:], invq[:])
            selfexp = sm.tile([P, MT], f32, tag="selfexp")
            nc.scalar.activation(selfexp[:], self_t[:], AF.Exp, scale=INV_T)

            # pos_t = posr * invq * invp / t
            pos_t = sm.tile([P, MT], f32, tag="pos_t")
            nc.vector.tensor_mul(pos_t[:], posr[:], invq[:])
            nc.vector.tensor_mul(pos_t[:], pos_t[:], invp[:])
            nc.vector.tensor_scalar_mul(pos_t[:], pos_t[:], INV_T)

            if dbg:
                nc.gpsimd.dma_start(dbg_d["n2q"][:], n2q[:])
                nc.gpsimd.dma_start(dbg_d["invq"][:], invq[:])
                nc.gpsimd.dma_start(dbg_d["pos_t"][:], pos_t[:])
                nc.gpsimd.dma_start(dbg_d["selfexp"][:], selfexp[:])

            # ---- persistent xt tiles + per-group pipeline ----
            xt_sb = [xp.tile([P, two_n], f32r, tag=f"xt{dt}", name=f"xt{dt}")
                     for dt in range(DT)]
            slots = sm.tile([P, MT * NT], f32, tag="slots")
            fu_parts = sm.tile([1, max(FT, 2)], f32, tag="fu_parts")

            for g in range(NT):
                gs = slice(g * 512, (g + 1) * 512)
                # load
                for dt in range(DT):
                    nc.gpsimd.dma_start(xt_sb[dt][:, gs], xt_d[dt * P:(dt + 1) * P, gs])
                # column norms^2 of this group
                ps_n2 = pss.tile([1, 512], f32, tag="small")
                for dt in range(DT):
                    sq = sqp.tile([P, 512], f32r, tag="sq")
                    nc.vector.tensor_mul(sq[:], xt_sb[dt][:, gs], xt_sb[dt][:, gs])
                    nc.tensor.matmul(ps_n2[:], ones_col[:], sq[:],
                                     start=(dt == 0), stop=(dt == DT - 1))
                # inv = 1/sqrt(n2) on a [1,512] piece
                rp = rowp.tile([1, 512], f32, tag="rp")
                nc.scalar.activation(rp[:], ps_n2[:], AF.Sqrt)
                ri = rowp.tile([1, 512], f32r, tag="ri")
                with nc.allow_low_precision(reason="f32r is storage-identical to f32"):
                    nc.vector.reciprocal(ri[:], rp[:])
                # broadcast inv across partitions via K=1 matmul
                ps_b = psb.tile([P, 512], f32, tag="bc")
                nc.tensor.matmul(ps_b[:], ones_row[:], ri[:], start=True, stop=True)
                bc = bcp.tile([P, 512], f32r, tag="bc")
                nc.scalar.activation(bc[:], ps_b[:], AF.Copy)
                if dbg and g == 0:
                    nc.gpsimd.dma_start(dbg_d["bc0"][:], bc[:])
                # normalize columns in place
                for dt in range(DT):
                    nc.vector.tensor_mul(xt_sb[dt][:, gs], xt_sb[dt][:, gs], bc[:])

                # GEMM block: all m-tiles against this column group
                for mt in range(MT):
                    ps = psg.tile([P, 512], f32, tag="gemm")
                    for dt in range(DT):
                        nc.tensor.matmul(
                            ps[:],
                            qt_sb[dt][:, mt * P:(mt + 1) * P],
                            xt_sb[dt][:, gs],
                            start=(dt == 0), stop=(dt == DT - 1))
                    scr = scrp.tile([P, 512], f32, tag="scr")
                    nc.scalar.activation(
                        scr[:], ps[:], AF.Exp,
                        scale=invq_t[:, mt:mt + 1],
                        accum_out=slots[:, mt * NT + g:mt * NT + g + 1])
                    if dbg and g == 0 and mt == 0:
                        nc.gpsimd.dma_start(dbg_d["g00"][:], scr[:])

                # fu slice (cols g*512..) while zt_i columns are fresh
                if g < FT:
                    ps_k = pss.tile([1, 512], f32, tag="small")
                    ps_f = pss.tile([1, 512], f32, tag="small")
                    for dt in range(DT):
                        ktc = stream.tile([P, 512], f32r, tag="kt")
                        nc.gpsimd.dma_start(ktc[:], kt_d[dt * P:(dt + 1) * P, gs])
                        sqk = sqp.tile([P, 512], f32r, tag="sq")
                        nc.vector.tensor_mul(sqk[:], ktc[:], ktc[:])
                        nc.tensor.matmul(ps_k[:], ones_col[:], sqk[:],
                                         start=(dt == 0), stop=(dt == DT - 1))
                        fk = sqp.tile([P, 512], f32r, tag="sq")
                        nc.vector.tensor_mul(fk[:], ktc[:], xt_sb[dt][:, gs])
                        nc.tensor.matmul(ps_f[:], ones_col[:], fk[:],
                                         start=(dt == 0), stop=(dt == DT - 1))
                    kp = rowp.tile([1, 512], f32, tag="rp")
                    nc.scalar.activation(kp[:], ps_k[:], AF.Sqrt)
                    ki = rowp.tile([1, 512], f32, tag="ri")
                    nc.vector.reciprocal(ki[:], kp[:])
                    fp = rowp.tile([1, 512], f32, tag="fp")
                    nc.vector.tensor_mul(fp[:], ps_f[:], ki[:])
                    nc.scalar.activation(fp[:], fp[:], AF.Exp, scale=INV_T)
                    nc.vector.reduce_sum(fu_parts[0:1, g:g + 1], fp[:], axis=AX.X)

            # ---- fu scalar -> broadcast [P,1] via DRAM bounce ----
            fu_tot = sm.tile([1, 1], f32, tag="fu_tot")
            nc.vector.reduce_sum(fu_tot[:], fu_parts[0:1, 0:FT], axis=AX.X)
            nc.vector.tensor_scalar_mul(fu_tot[:], fu_tot[:], 2.0)
            nc.gpsimd.dma_start(fu_scr[:], fu_tot[:])
            fu_bc = sm.tile([P, 1], f32, tag="fu_bc")
            fu_bcast_ap = bass.AP(tensor=fu_scr[:].tensor, offset=0, ap=[[0, P], [1, 1]])
            nc.gpsimd.dma_start(fu_bc[:], fu_bcast_ap)

            # ---- assemble per-row loss partials ----
            rs_all = sm.tile([P, MT], f32, tag="rs_all")
            for mt in range(MT):
                nc.vector.reduce_sum(
                    rs_all[:, mt:mt + 1], slots[:, mt * NT:(mt + 1) * NT], axis=AX.X)
            denom = sm.tile([P, MT], f32, tag="denom")
            nc.vector.scalar_tensor_tensor(
                denom[:], rs_all[:], fu_bc[:], selfexp[:], OP.add, OP.subtract)
            if dbg:
                nc.gpsimd.dma_start(dbg_d["slots"][:], slots[:])
                nc.gpsimd.dma_start(dbg_d["rs_all"][:], rs_all[:])
                nc.gpsimd.dma_start(dbg_d["denom"][:], denom[:])
                nc.gpsimd.dma_start(dbg_d["fu_parts"][:], fu_parts[:])
            lnd = sm.tile([P, MT], f32, tag="lnd")
            ln_sum = sm.tile([P, 1], f32, tag="ln_sum")
            nc.scalar.activation(lnd[:], denom[:], AF.Ln, accum_out=ln_sum[:])
            possum = sm.tile([P, 1], f32, tag="possum")
            nc.vector.reduce_sum(possum[:], pos_t[:], axis=AX.X)
            total = sm.tile([P, 1], f32, tag="total")
            nc.vector.tensor_sub(total[:], ln_sum[:], possum[:])
            nc.gpsimd.dma_start(out_d[:], total[:])

    nc.finalize()
    return nc


def shard_inputs(emb_i, emb_j, emb_k, n_cores=8):
    """Host-side sharding: build the per-core input maps."""
    two_n = emb_i.shape[0] * 2
    q = two_n // n_cores
    n = two_n // 2
    X = np.concatenate([emb_i, emb_j], axis=0)
    xt = np.ascontiguousarray(X.T, dtype=np.float32)
    kt = np.ascontiguousarray(emb_k.T, dtype=np.float32)
    ones = np.ones((128, 128), dtype=np.float32)
    in_maps = []
    for c in range(n_cores):
        q0 = c * q
        p0 = (q0 + n) % two_n
        in_maps.append({
            "xt": xt,
            "qt": np.ascontiguousarray(xt[:, q0:q0 + q]),
            "pt": np.ascontiguousarray(xt[:, p0:p0 + q]),
            "kt": kt,
            "ones": ones,
        })
    return in_maps


_NC_CACHE = {}


def _get_nc(key=(8192, 512, 1024, 4096)):
    if key not in _NC_CACHE:
        _NC_CACHE[key] = build_nc(*key)
    return _NC_CACHE[key]


def kernel(emb_i, emb_j, emb_k):
    from concourse.bass_utils import run_bass_kernel_spmd

    n_cores = 8
    in_maps = shard_inputs(emb_i, emb_j, emb_k, n_cores)
    nc = _get_nc()
    res = run_bass_kernel_spmd(nc, in_maps, list(range(n_cores))).results
    total = sum(float(np.sum(r["partial"].astype(np.float64))) for r in res)
    two_n = emb_i.shape[0] * 2
    return np.asarray(np.float32(total / two_n))



# revision 4
# speedup vs baseline: 1.3280x; 1.3280x over previous
"""Trainium2 Bass kernel for nn_ContrastiveLoss (B=4096, D=512, 8 cores).

Strategy (data-parallel over the 2B=8192 rows of reps = [emb_i; emb_j]):
  - Host passes each core a ROTATED X.T (bf16, own 1024 columns always at
    position 0, partner block always at 4096) so the program is SPMD-clean,
    plus a per-core 512-column slice of emb_k.T / emb_i.T for the fu term.
  - On device: column norms via bf16 squares (DVE 2x) + ones-vector matmuls
    into slices of the same rotating [128,2048] PSUM pool the GEMM uses;
    rsqrt computed as exp(-0.5*ln(n2)) on a [128,16]-packed tile (single
    ActE table for Exp/Ln/Copy - no table swaps), broadcast back to a
    [128,8192] column-scale tile via stride-0-partition DMA.
  - Main GEMM: z.T @ z in bf16, quarter-pipelined (2048-col quarters), wide
    [128,2048] PSUM tiles, exp fused on ScalarE with accum_out row-sums.
  - Self-similarity term is exactly exp(1/t) = e^5: subtracted as constant.
  - fu (rowwise dot(z_k, z_i)) sharded 512 cols/core, combined with an
    8-core AllReduce that overlaps the GEMM.
  - Positive-pair sums extracted as one scalar per core via elementwise
    mul + ones-matmul on the normalized tiles.
"""

import numpy as np
import ml_dtypes

import concourse.bass as bass
import concourse.mybir as mybir
import concourse.tile as tile
from concourse import bacc

f32 = mybir.dt.float32
bf16 = mybir.dt.bfloat16
AF = mybir.ActivationFunctionType
OP = mybir.AluOpType
AX = mybir.AxisListType

P = 128
TEMP = 0.2
INV_T = 1.0 / TEMP  # 5.0
E5 = float(np.exp(5.0))  # self-similarity exp(1/t), z.z == 1

TWO_N = 8192
D = 512
DT = D // P            # 4 contraction tiles
Q = 1024               # rows per core
MT = Q // P            # 8 output row tiles
QW = 2048              # quarter width (cols)
NQ = TWO_N // QW       # 4 quarters
G = 512                # psum bank slice width
GPQ = QW // G          # 4 groups per quarter
FU = 512               # fu columns per core


def build_nc(use_cc=True, dbg=False):
    nc = bacc.Bacc("TRN2", target_bir_lowering=False, debug=False,
                   num_devices=8)

    xt_d = nc.dram_tensor("xt", [D, TWO_N], bf16, kind="ExternalInput")
    kt_d = nc.dram_tensor("kt", [D, FU], bf16, kind="ExternalInput")
    xi_d = nc.dram_tensor("xi", [D, FU], bf16, kind="ExternalInput")
    ones_d = nc.dram_tensor("ones", [P, P], bf16, kind="ExternalInput")
    ln_out = nc.dram_tensor("lnsum", [P, 1], f32, kind="ExternalOutput")
    pos_out = nc.dram_tensor("postot", [1, 1], f32, kind="ExternalOutput")
    fu_out = nc.dram_tensor("fuout", [1, 1], f32, kind="ExternalOutput")

    n2_d = nc.dram_tensor("n2_scr", [1, TWO_N], f32)
    inv_d = nc.dram_tensor("inv_scr", [1, TWO_N], bf16)
    fu_cc = nc.dram_tensor("fu_cc", [1, 16], f32)
    fuadj_d = nc.dram_tensor("fuadj_scr", [1, 1], f32)
    if dbg:
        dbg_d = {name: nc.dram_tensor(f"dbg_{name}", shape, dt, kind="ExternalOutput")
                 for name, shape, dt in [
                     ("n2", [1, TWO_N], f32), ("inv", [1, TWO_N], bf16),
                     ("bc0", [P, G], bf16), ("xtn0", [P, G], bf16),
                     ("slots", [P, MT * NQ], f32), ("rs", [P, MT], f32),
                     ("denom", [P, MT], f32), ("fu16", [1, 16], f32),
                 ]}

    with tile.TileContext(nc) as tc:
        with (
            tc.tile_pool(name="xp", bufs=1) as xp,       # persistent xt tiles
            tc.tile_pool(name="bcp", bufs=1) as bcp,     # column scale bcast
            tc.tile_pool(name="sqp", bufs=3) as sqp,     # square chunks
            tc.tile_pool(name="scrp", bufs=2) as scrp,   # exp outputs
            tc.tile_pool(name="fup", bufs=1) as fup,     # kt/xi tiles
            tc.tile_pool(name="sm", bufs=1) as sm,       # persistent smalls
            tc.tile_pool(name="ps", bufs=2, space="PSUM") as psg,  # 2x4 banks
        ):
            ones_col = sm.tile([P, 1], bf16, tag="ones_col")
            nc.gpsimd.dma_start(ones_col[:], ones_d[:, 0:1])

            # ---- input DMAs: kt/xi first (small), then xt quarter-major ----
            kts, xis = [], []
            for dt in range(DT):
                kt_t = fup.tile([P, FU], bf16, tag=f"kt{dt}")
                nc.gpsimd.dma_start(kt_t[:], kt_d[dt * P:(dt + 1) * P, :])
                kts.append(kt_t)
                xi_t = fup.tile([P, FU], bf16, tag=f"xi{dt}")
                nc.gpsimd.dma_start(xi_t[:], xi_d[dt * P:(dt + 1) * P, :])
                xis.append(xi_t)

            xt_sb = [xp.tile([P, TWO_N], bf16, tag=f"xt{dt}", name=f"xt{dt}")
                     for dt in range(DT)]
            for q in range(NQ):
                for dt in range(DT):
                    for j in range(GPQ):
                        c0 = q * QW + j * G
                        nc.gpsimd.dma_start(
                            xt_sb[dt][:, c0:c0 + G],
                            xt_d[dt * P:(dt + 1) * P, c0:c0 + G])

            bc = bcp.tile([P, TWO_N], bf16, tag="bc")
            slots = sm.tile([P, MT * NQ], f32, tag="slots")
            fu16 = sm.tile([1, 16], f32, tag="fu16")
            nc.vector.memset(fu16[:], 0.0)

            # ================= fu block (one PSUM wide buf) =================
            ps_fu = psg.tile([P, QW], f32, tag="wide")
            for dt in range(DT):
                sqk = sqp.tile([P, QW], bf16, tag="sq")
                nc.vector.tensor_mul(sqk[:, 0:FU], kts[dt][:], kts[dt][:])
                nc.tensor.matmul(ps_fu[0:1, 0:FU], ones_col[:], sqk[:, 0:FU],
                                 start=(dt == 0), stop=(dt == DT - 1))
                sqi = sqp.tile([P, QW], bf16, tag="sq")
                nc.vector.tensor_mul(sqi[:, 0:FU], xis[dt][:], xis[dt][:])
                nc.tensor.matmul(ps_fu[0:1, FU:2 * FU], ones_col[:], sqi[:, 0:FU],
                                 start=(dt == 0), stop=(dt == DT - 1))
                dki = sqp.tile([P, QW], bf16, tag="sq")
                nc.vector.tensor_mul(dki[:, 0:FU], kts[dt][:], xis[dt][:])
                nc.tensor.matmul(ps_fu[0:1, 2 * FU:3 * FU], ones_col[:], dki[:, 0:FU],
                                 start=(dt == 0), stop=(dt == DT - 1))
            lnk = sm.tile([1, FU], f32, tag="lnk")
            nc.scalar.activation(lnk[:], ps_fu[0:1, 0:FU], AF.Ln)
            lni = sm.tile([1, FU], f32, tag="lni")
            nc.scalar.activation(lni[:], ps_fu[0:1, FU:2 * FU], AF.Ln)
            lsum = sm.tile([1, FU], f32, tag="lsum")
            nc.vector.tensor_add(lsum[:], lnk[:], lni[:])
            inv_ki = sm.tile([1, FU], f32, tag="inv_ki")
            nc.scalar.activation(inv_ki[:], lsum[:], AF.Exp, scale=-0.5)
            fvals = sm.tile([1, FU], f32, tag="fvals")
            nc.vector.tensor_mul(fvals[:], ps_fu[0:1, 2 * FU:3 * FU], inv_ki[:])
            fscr = sm.tile([1, FU], f32, tag="fscr")
            nc.scalar.activation(fscr[:], fvals[:], AF.Exp, scale=INV_T,
                                 accum_out=fu16[0:1, 0:1])
            nc.gpsimd.dma_start(fu_cc[:], fu16[:])
            if use_cc:
                nc.gpsimd.collective_compute(
                    "AllReduce", OP.add,
                    replica_groups=[[i for i in range(8)]],
                    ins=[fu_cc[:].opt()], outs=[fu_cc[:].opt()])
            # fuadj = 2*fu_total - e^5, broadcast to [P,1]
            fu_ret = sm.tile([1, 1], f32, tag="fu_ret")
            nc.gpsimd.dma_start(fu_ret[:], fu_cc[0:1, 0:1])
            fuadj = sm.tile([1, 1], f32, tag="fuadj")
            nc.vector.tensor_scalar(fuadj[:], fu_ret[:], 2.0, -E5,
                                    OP.mult, OP.add)
            nc.gpsimd.dma_start(fu_out[:], fuadj[:])
            nc.gpsimd.dma_start(fuadj_d[:], fuadj[:])
            fuadj_bc = sm.tile([P, 1], f32, tag="fuadj_bc")
            bc_ap = bass.AP(tensor=fuadj_d[:].tensor, offset=0,
                            ap=[[0, P], [1, 1]])
            nc.gpsimd.dma_start(fuadj_bc[:], bc_ap)
            if dbg:
                nc.gpsimd.dma_start(dbg_d["fu16"][:], fu16[:])

            # ============ per-quarter: reduce -> inv -> bcast -> norm =======
            def emit_norm_quarter(q):
                c0 = q * QW
                ps_n2 = psg.tile([P, QW], f32, tag="wide")
                for j in range(GPQ):
                    for dt in range(DT):
                        sq = sqp.tile([P, QW], bf16, tag="sq")
                        s = slice(c0 + j * G, c0 + (j + 1) * G)
                        nc.vector.tensor_mul(sq[:, 0:G], xt_sb[dt][:, s],
                                             xt_sb[dt][:, s])
                        nc.tensor.matmul(ps_n2[0:1, j * G:(j + 1) * G],
                                         ones_col[:], sq[:, 0:G],
                                         start=(dt == 0), stop=(dt == DT - 1))
                # drain psum -> sbuf (DVE) -> dram
                n2row = sm.tile([1, QW], f32, tag=f"n2row{q % 2}")
                nc.vector.tensor_scalar_mul(n2row[:], ps_n2[0:1, :], 1.0)
                nc.gpsimd.dma_start(n2_d[0:1, c0:c0 + QW], n2row[:])
                # pack [1,2048] -> [128,16], rsqrt = exp(-0.5*ln), unpack
                n2p = sm.tile([P, QW // P], f32, tag=f"n2p{q}")
                nc.gpsimd.dma_start(
                    n2p[:], n2_d[0:1, c0:c0 + QW].rearrange(
                        "a (p f) -> (a p) f", p=P))
                lnp = sm.tile([P, QW // P], f32, tag=f"lnp{q}")
                nc.scalar.activation(lnp[:], n2p[:], AF.Ln)
                invp = sm.tile([P, QW // P], bf16, tag=f"invp{q}")
                nc.scalar.activation(invp[:], lnp[:], AF.Exp, scale=-0.5)
                nc.gpsimd.dma_start(
                    inv_d[0:1, c0:c0 + QW].rearrange("a (p f) -> (a p) f", p=P),
                    invp[:])
                # broadcast inv row across partitions into bc (4 chunks)
                for j in range(GPQ):
                    sl = inv_d[0:1, c0 + j * G:c0 + (j + 1) * G]
                    rep = bass.AP(tensor=sl.tensor, offset=sl.offset,
                                  ap=[[0, P]] + list(sl.ap)[1:])
                    nc.gpsimd.dma_start(bc[:, c0 + j * G:c0 + (j + 1) * G], rep)
                # normalize in place
                for dt in range(DT):
                    nc.vector.tensor_mul(xt_sb[dt][:, c0:c0 + QW],
                                         xt_sb[dt][:, c0:c0 + QW],
                                         bc[:, c0:c0 + QW])

            # ============ GEMM for one quarter ============
            def emit_gemm_quarter(q):
                c0 = q * QW
                for mt in range(MT):
                    ps = psg.tile([P, QW], f32, tag="wide")
                    for dt in range(DT):
                        for j in range(GPQ):
                            s = slice(c0 + j * G, c0 + (j + 1) * G)
                            nc.tensor.matmul(
                                ps[:, j * G:(j + 1) * G],
                                xt_sb[dt][:, mt * P:(mt + 1) * P],
                                xt_sb[dt][:, s],
                                start=(dt == 0), stop=(dt == DT - 1))
                    scr = scrp.tile([P, QW], bf16, tag="scr")
                    nc.scalar.activation(
                        scr[:], ps[:], AF.Exp, scale=INV_T,
                        accum_out=slots[:, mt * NQ + q:mt * NQ + q + 1])

            emit_norm_quarter(0)
            for q in range(NQ):
                if q + 1 < NQ:
                    emit_norm_quarter(q + 1)
                emit_gemm_quarter(q)

            # ============ positives: sum over rows of z_q . z_p ============
            ps_pos = psg.tile([P, QW], f32, tag="wide")
            k = 0
            for gg in range(Q // G):
                for dt in range(DT):
                    pp = sqp.tile([P, QW], bf16, tag="sq")
                    nc.vector.tensor_mul(
                        pp[:, 0:G], xt_sb[dt][:, gg * G:(gg + 1) * G],
                        xt_sb[dt][:, 4096 + gg * G:4096 + (gg + 1) * G])
                    nc.tensor.matmul(ps_pos[0:1, 0:G], ones_col[:], pp[:, 0:G],
                                     start=(k == 0), stop=(k == Q // G * DT - 1))
                    k += 1
            postot = sm.tile([1, 1], f32, tag="postot")
            nc.vector.reduce_sum(postot[:], ps_pos[0:1, 0:G], axis=AX.X)
            nc.gpsimd.dma_start(pos_out[:], postot[:])

            # ============ per-row denominators and log-sum ============
            rs = sm.tile([P, MT], f32, tag="rs")
            for mt in range(MT):
                nc.vector.reduce_sum(rs[:, mt:mt + 1],
                                     slots[:, mt * NQ:(mt + 1) * NQ], axis=AX.X)
            denom = sm.tile([P, MT], f32, tag="denom")
            nc.vector.tensor_scalar_add(denom[:], rs[:], fuadj_bc[:])
            lnd = sm.tile([P, MT], f32, tag="lnd")
            lns = sm.tile([P, 1], f32, tag="lns")
            nc.scalar.activation(lnd[:], denom[:], AF.Ln, accum_out=lns[:])
            nc.gpsimd.dma_start(ln_out[:], lns[:])
            if dbg:
                nc.gpsimd.dma_start(dbg_d["n2"][:], n2_d[:])
                nc.gpsimd.dma_start(dbg_d["inv"][:], inv_d[:])
                nc.gpsimd.dma_start(dbg_d["bc0"][:], bc[:, 0:G])
                nc.gpsimd.dma_start(dbg_d["xtn0"][:], xt_sb[0][:, 0:G])
                nc.gpsimd.dma_start(dbg_d["slots"][:], slots[:])
                nc.gpsimd.dma_start(dbg_d["rs"][:], rs[:])
                nc.gpsimd.dma_start(dbg_d["denom"][:], denom[:])

    nc.finalize()
    return nc


def shard_inputs(emb_i, emb_j, emb_k, n_cores=8):
    """Host-side sharding: rotate columns so each core's own block is at 0."""
    X = np.concatenate([emb_i, emb_j], axis=0)
    xt = np.ascontiguousarray(X.T).astype(ml_dtypes.bfloat16)
    kt = np.ascontiguousarray(emb_k.T).astype(ml_dtypes.bfloat16)
    xit = xt[:, :4096]
    ones = np.ones((P, P), dtype=ml_dtypes.bfloat16)
    in_maps = []
    for c in range(n_cores):
        q0 = c * Q
        in_maps.append({
            "xt": np.ascontiguousarray(np.roll(xt, -q0, axis=1)),
            "kt": np.ascontiguousarray(kt[:, c * FU:(c + 1) * FU]),
            "xi": np.ascontiguousarray(xit[:, c * FU:(c + 1) * FU]),
            "ones": ones,
        })
    return in_maps


def combine_results(results, two_n=TWO_N):
    total = 0.0
    for r in results:
        total += float(np.sum(r["lnsum"].astype(np.float64)))
        total -= INV_T * float(r["postot"].reshape(-1)[0])
    return np.asarray(np.float32(total / two_n))


_NC_CACHE = {}


def _get_nc(key="v1"):
    if key not in _NC_CACHE:
        _NC_CACHE[key] = build_nc()
    return _NC_CACHE[key]


def kernel(emb_i, emb_j, emb_k):
    from concourse.bass_utils import run_bass_kernel_spmd

    n_cores = 8
    in_maps = shard_inputs(emb_i, emb_j, emb_k, n_cores)
    nc = _get_nc()
    res = run_bass_kernel_spmd(nc, in_maps, list(range(n_cores))).results
    return combine_results(res)
